# revision 1
# baseline (speedup 1.0000x reference)
"""GPT-OSS transformer block on 8 Trainium2 NeuronCores (Bass/Tile).

Sharding: sequence-parallel attention (each core owns 4 interleaved
128-token chunks of one batch; KV duplicated within a batch group),
expert-parallel MoE (core c owns expert c; tokens routed on-chip via
GpSimd sparse_gather compaction + indirect-DMA row gather/scatter;
AllGather for token features, ReduceScatter for the combine).

Compute dtype bf16 with fp32 PSUM accumulation and fp32 residuals.
One NEFF shared by all 8 cores (SPMD): all loop structure is static
(worst-case causal padding); per-core variation enters only via data
(weight shards, host-sliced columns, multiplicative causal masks,
expert id scalar).
"""

import sys
import types
import functools

import numpy as np
import ml_dtypes

B, S, D = 2, 2048, 1024
H, HKV, HD = 16, 4, 64
F, E = 2048, 8
T = B * S
N_CORES = 8
THETA = 10000.0
EPS = 1e-5
CAPH = 640                    # slots per expert per local-half (max real 583)
NSTH = CAPH // 128            # 5 slot tiles per half
SCH_H = [(0, 512), (512, 128)]  # slot chunks per half
DT = D // 128                 # 8 d k-tiles
FT = F // 128                 # 16 f tiles
NCH = S // 128                # 16 k-chunks per batch
GP = [3, 7, 11, 15]           # padded (worst-case) slot -> global chunk map

bf16 = ml_dtypes.bfloat16

DBG = False                   # adds debug outputs (h2T, hnT, logitsN)


def _hook_init():
    """Inject antenv.axon_hooks (absent in this image) so trace=True works."""
    try:
        import antenv
    except ImportError:
        return
    if hasattr(antenv, "axon_hooks"):
        return
    m = types.ModuleType("antenv.axon_hooks")
    m._hook = None
    def _set(h): m._hook = h
    def _get(): return m._hook
    m.set_axon_ntff_profile_hook = _set
    m.get_axon_ntff_profile_hook = _get
    sys.modules["antenv.axon_hooks"] = m
    antenv.axon_hooks = m
    try:
        from trn_agent_boot.trn_boot import _ntff_profile_via_ctypes
        hook = _ntff_profile_via_ctypes("/opt/axon/libaxon_pjrt.so")
        if hook is not None:
            _set(hook)
    except Exception:
        pass


def build():
    import concourse.bass as bass
    import concourse.tile as tile
    from concourse import bacc, mybir

    f32 = mybir.dt.float32
    b16 = mybir.dt.bfloat16
    i32 = mybir.dt.int32
    u32 = mybir.dt.uint32
    AF = mybir.ActivationFunctionType
    ALU = mybir.AluOpType

    nc = bacc.Bacc("TRN2", target_bir_lowering=False, debug=False,
                   num_devices=N_CORES)

    def din(name, shape, dt=b16):
        return nc.dram_tensor(name, shape, dt, kind="ExternalInput")

    DA = D + 8      # augmented hn row: 1024 features + e1,e2,w1,w2 (bf16) + pad

    xT = din("xT", [D, S])
    xTq = din("xTq", [D, 512])
    xTr = din("xTr", [D, 512], f32)
    wq = din("wq", [D, D])
    wk = din("wk", [D, 256])
    wv = din("wv", [D, 256])
    wo = din("wo", [D, D])
    wr = din("wr", [D, E])
    wg = din("wg", [D, F])
    wu = din("wu", [D, F])
    wd = din("wd", [F, D])
    cosq = din("cosq", [128, 512])
    sinq = din("sinq", [128, 512])
    cosk = din("cosk", [128, S])
    sink = din("sink", [128, S])
    maskm = din("maskm", [4, 4, 128, 128])
    identb = din("identb", [128, 128])
    identv = din("identv", [128, 64])
    identf = din("identf", [128, 128], f32)
    onesb = din("onesb", [128, 1])
    eid = din("eid", [16, 1], f32)
    hA = din("hA", [16, 1], f32)                # 1 if partition even (local tok < 256)
    hAm1 = din("hAm1", [16, 1], f32)            # hA - 1
    hB = din("hB", [16, 1], f32)
    hBm1 = din("hBm1", [16, 1], f32)

    out = nc.dram_tensor("out", [512, D], f32, kind="ExternalOutput")
    dbg = {}
    if DBG:
        dbg["h2T"] = nc.dram_tensor("dbg_h2T", [D, 512], f32, kind="ExternalOutput")
        dbg["vals"] = nc.dram_tensor("dbg_vals", [16, 256], f32, kind="ExternalOutput")
        dbg["cnt"] = nc.dram_tensor("dbg_cnt", [1, 1], u32, kind="ExternalOutput")
        dbg["part"] = nc.dram_tensor("dbg_part", [T, D], b16, kind="ExternalOutput")
        dbg["rk"] = nc.dram_tensor("dbg_rk", [128, S], f32, kind="ExternalOutput")
        dbg["hnall"] = nc.dram_tensor("dbg_hnall", [T, DA], b16, kind="ExternalOutput")

    hn_bounce = nc.dram_tensor("hn_bounce", [512, DA], b16)
    hn_all = nc.dram_tensor("hn_all", [T + 8, DA], b16, addr_space="Shared")
    partA = nc.dram_tensor("partA", [T // 2 + 8, D], b16)
    partB = nc.dram_tensor("partB", [T // 2 + 8, D], b16)
    rsA = nc.dram_tensor("rsA", [256, D], b16)
    rsB = nc.dram_tensor("rsB", [256, D], b16)
    rstd_scr = nc.dram_tensor("rstd_scr", [16, 128], f32)
    rstdq_scr = nc.dram_tensor("rstdq_scr", [4, 128], f32)
    rstd2_scr = nc.dram_tensor("rstd2_scr", [4, 128], f32)
    den_scr = nc.dram_tensor("den_scr", [H, 4, 128], f32)

    RG = [list(range(N_CORES))]
    PERM_HEADS = [0, 4, 1, 5, 2, 6, 3, 7, 8, 12, 9, 13, 10, 14, 11, 15]
    POS = [PERM_HEADS.index(h) for h in range(H)]

    def bcast_row(bc_ap, dram, width, parts, base_off=0):
        # bc[q, t] = dram_flat[base_off + t] for all q (contiguous free dim)
        nc.sync.dma_start(
            out=bc_ap,
            in_=bass.AP(tensor=dram.ap().tensor, offset=base_off,
                        ap=[[0, parts], [1, width]]))

    with tile.TileContext(nc) as tc:
        with tc.tile_pool(name="const", bufs=1) as cpool, \
             tc.tile_pool(name="persist", bufs=1) as ppool:

            ident = cpool.tile([128, 128], b16)
            nc.sync.dma_start(out=ident[:], in_=identb.ap())
            identv_t = cpool.tile([128, 64], b16)
            nc.sync.dma_start(out=identv_t[:], in_=identv.ap())
            identf_t = cpool.tile([128, 128], f32)
            nc.sync.dma_start(out=identf_t[:], in_=identf.ap())
            ones = cpool.tile([128, 1], b16)
            nc.sync.dma_start(out=ones[:], in_=onesb.ap())
            CQ = cpool.tile([128, 512], b16)
            nc.sync.dma_start(out=CQ[:], in_=cosq.ap())
            SQ = cpool.tile([128, 512], b16)
            nc.sync.dma_start(out=SQ[:], in_=sinq.ap())
            mm_t = [[cpool.tile([128, 128], b16, tag=f"mm{j}{r}", name=f"mm{j}{r}")
                     for r in range(4)] for j in range(4)]
            for j in range(4):
                for r in range(4):
                    nc.sync.dma_start(out=mm_t[j][r][:], in_=maskm.ap()[j, r])

            zrow = cpool.tile([128, DA], b16)
            nc.vector.memset(zrow[:], 0)
            for i in range(T // 256):
                nc.sync.dma_start(out=partA.ap()[128 * i:128 * (i + 1)],
                                  in_=zrow[:, :D])
                nc.sync.dma_start(out=partB.ap()[128 * i:128 * (i + 1)],
                                  in_=zrow[:, :D])
            nc.sync.dma_start(out=hn_all.ap()[T:T + 8], in_=zrow[:8, :])

            h2T = [ppool.tile([128, 512], f32, tag=f"h2T{kt}", name=f"h2T{kt}")
                   for kt in range(DT)]
            h2N = [ppool.tile([128, D], b16, tag=f"h2N{tc_}", name=f"h2N{tc_}")
                   for tc_ in range(4)]

            # ================= attention =================
            with tc.tile_pool(name="attn", bufs=1) as ap_, \
                 tc.tile_pool(name="atmp", bufs=3) as tmp, \
                 tc.tile_pool(name="ptp", bufs=6) as ptp:

                QT = [ap_.tile([128, 512], b16, tag=f"QT{m}", name=f"QT{m}")
                      for m in range(DT)]
                KT_ = [ap_.tile([128, S], b16, tag=f"KT{m}", name=f"KTm{m}")
                       for m in range(2)]
                VN = [ap_.tile([128, 4 * 65], b16, tag=f"VN{kc}", name=f"VN{kc}")
                      for kc in range(NCH)]
                CTX = [ap_.tile([128, 512], b16, tag=f"CTX{m}", name=f"CTX{m}")
                       for m in range(DT)]
                XQ = [ap_.tile([128, 512], b16, tag=f"XQ{kt}", name=f"XQ{kt}")
                      for kt in range(DT)]
                for kt in range(DT):
                    nc.sync.dma_start(out=XQ[kt][:],
                                      in_=xTq.ap()[128 * kt:128 * (kt + 1)])

                def rope_evac(psum, width, cos_t, sin_t, rbc, out_ap):
                    raw = tmp.tile([128, 512], b16, tag="raw", name="raw")
                    nc.vector.tensor_tensor(out=raw[:, :width], in0=psum[:, :width],
                                            in1=rbc[:, :width], op=ALU.mult)
                    rot = tmp.tile([128, 512], b16, tag="rot", name="rot")
                    for hf in range(4):
                        src = hf * 32 + (32 if hf % 2 == 0 else -32)
                        nc.vector.tensor_copy(out=rot[32 * hf:32 * (hf + 1), :width],
                                              in_=raw[src:src + 32, :width])
                    t1 = tmp.tile([128, 512], b16, tag="t1", name="t1")
                    nc.vector.tensor_tensor(out=t1[:, :width], in0=raw[:, :width],
                                            in1=cos_t[:, :width], op=ALU.mult)
                    nc.vector.tensor_tensor(out=rot[:, :width], in0=rot[:, :width],
                                            in1=sin_t[:, :width], op=ALU.mult)
                    nc.vector.tensor_tensor(out=out_ap, in0=t1[:, :width],
                                            in1=rot[:, :width], op=ALU.add)

                with tc.tile_pool(name="qkvs", bufs=1) as qp:
                    XT = [qp.tile([128, S], b16, tag=f"XT{kt}", name=f"XT{kt}")
                          for kt in range(DT)]
                    WQ = [qp.tile([128, D], b16, tag=f"WQ{kt}", name=f"WQ{kt}")
                          for kt in range(DT)]
                    WK = [qp.tile([128, 256], b16, tag=f"WK{kt}", name=f"WK{kt}")
                          for kt in range(DT)]
                    WV = [qp.tile([128, 256], b16, tag=f"WV{kt}", name=f"WV{kt}")
                          for kt in range(DT)]
                    for kt in range(DT):
                        nc.sync.dma_start(out=XT[kt][:],
                                          in_=xT.ap()[128 * kt:128 * (kt + 1)])
                        nc.sync.dma_start(out=WQ[kt][:],
                                          in_=wq.ap()[128 * kt:128 * (kt + 1)])
                        nc.sync.dma_start(out=WK[kt][:],
                                          in_=wk.ap()[128 * kt:128 * (kt + 1)])
                        nc.sync.dma_start(out=WV[kt][:],
                                          in_=wv.ap()[128 * kt:128 * (kt + 1)])
                    CK = qp.tile([128, S], b16)
                    nc.sync.dma_start(out=CK[:], in_=cosk.ap())
                    SK = qp.tile([128, S], b16)
                    nc.sync.dma_start(out=SK[:], in_=sink.ap())

                    def rstd_of(src_tiles, width, scale_extra, tag, pspool, scr):
                        nt = width // 128
                        ssqP = pspool.tile([128, DT * nt], f32, tag=f"ssqP{tag}",
                                           name=f"ssqP{tag}")
                        for kt in range(DT):
                            x2 = tmp.tile([128, S], b16, tag="x2", name="x2")
                            nc.vector.tensor_tensor(out=x2[:, :width],
                                                    in0=src_tiles[kt][:],
                                                    in1=src_tiles[kt][:],
                                                    op=ALU.mult)
                            for j in range(nt):
                                nc.tensor.matmul(
                                    out=ssqP[:, kt * nt + j:kt * nt + j + 1],
                                    lhsT=x2[:, 128 * j:128 * (j + 1)],
                                    rhs=ones[:], start=True, stop=True)
                        rstdT = qp.tile([128, nt], f32, tag=f"rstdT{tag}",
                                        name=f"rstdT{tag}")
                        nc.vector.tensor_reduce(
                            out=rstdT[:],
                            in_=bass.AP(tensor=ssqP[:].tensor, offset=ssqP[:].offset,
                                        ap=[ssqP[:].ap[0], [1, nt], [nt, DT]]),
                            axis=mybir.AxisListType.X, op=ALU.add)
                        nc.vector.tensor_scalar(out=rstdT[:], in0=rstdT[:],
                                                scalar1=1.0 / D, scalar2=EPS,
                                                op0=ALU.mult, op1=ALU.add)
                        nc.scalar.sqrt(rstdT[:], rstdT[:])
                        nc.vector.reciprocal(rstdT[:], rstdT[:])
                        if scale_extra != 1.0:
                            nc.vector.tensor_scalar(out=rstdT[:], in0=rstdT[:],
                                                    scalar1=scale_extra,
                                                    scalar2=None, op0=ALU.mult)
                        rrp = pspool.tile([nt, 128], f32, tag=f"rrp{tag}",
                                          name=f"rrp{tag}")
                        nc.tensor.transpose(out=rrp[:], in_=rstdT[:],
                                            identity=identf_t[:])
                        rrs = qp.tile([nt, 128], f32, tag=f"rrs{tag}",
                                      name=f"rrs{tag}")
                        nc.vector.tensor_copy(out=rrs[:], in_=rrp[:])
                        nc.sync.dma_start(out=scr.ap(), in_=rrs[:])
                        bc = qp.tile([128, width], f32, tag=f"rstdbc{tag}",
                                     name=f"rstdbc{tag}")
                        bcast_row(bc[:], scr, width, 128)
                        return bc

                    with tc.tile_pool(name="psA", bufs=1, space="PSUM") as psA:
                        rstd_k = rstd_of(XT, S, 1.0, "k", psA, rstd_scr)
                        rstd_q = rstd_of(XQ, 512, 0.125, "q", psA, rstdq_scr)
                    if DBG:
                        nc.sync.dma_start(out=dbg["rk"].ap(), in_=rstd_k[:])

                    psB_cm = tc.tile_pool(name="psB", bufs=3, space="PSUM")
                    psB = psB_cm.__enter__()
                    psBt_cm = tc.tile_pool(name="psBt", bufs=2, space="PSUM")
                    psBt = psBt_cm.__enter__()

                    for m in range(DT):
                        q_ps = psB.tile([128, 512], f32, tag="qkv", name="q_ps")
                        for kt in range(DT):
                            nc.tensor.matmul(out=q_ps[:],
                                             lhsT=WQ[kt][:, 128 * m:128 * (m + 1)],
                                             rhs=XQ[kt][:], start=(kt == 0),
                                             stop=(kt == DT - 1))
                        rope_evac(q_ps, 512, CQ, SQ, rstd_q, QT[m][:])

                    for m in range(2):
                        for i in range(4):
                            k_ps = psB.tile([128, 512], f32, tag="qkv", name="k_ps")
                            for kt in range(DT):
                                nc.tensor.matmul(
                                    out=k_ps[:],
                                    lhsT=WK[kt][:, 128 * m:128 * (m + 1)],
                                    rhs=XT[kt][:, 512 * i:512 * (i + 1)],
                                    start=(kt == 0), stop=(kt == DT - 1))
                            rope_evac(k_ps, 512, CK[:, 512 * i:512 * (i + 1)],
                                      SK[:, 512 * i:512 * (i + 1)],
                                      rstd_k[:, 512 * i:512 * (i + 1)],
                                      KT_[m][:, 512 * i:512 * (i + 1)])

                    for kc in range(NCH):
                        nc.vector.memset(VN[kc][:], 0)
                    for m in range(2):
                        for i in range(4):
                            v_ps = psB.tile([128, 512], f32, tag="qkv", name="v_ps")
                            for kt in range(DT):
                                nc.tensor.matmul(
                                    out=v_ps[:],
                                    lhsT=WV[kt][:, 128 * m:128 * (m + 1)],
                                    rhs=XT[kt][:, 512 * i:512 * (i + 1)],
                                    start=(kt == 0), stop=(kt == DT - 1))
                            vt = tmp.tile([128, 512], b16, tag="vt", name="vt")
                            nc.vector.tensor_tensor(
                                out=vt[:], in0=v_ps[:],
                                in1=rstd_k[:, 512 * i:512 * (i + 1)], op=ALU.mult)
                            for kvh in range(2):
                                kv = 2 * m + kvh
                                for cc in range(4):
                                    kc = 4 * i + cc
                                    tp = psBt.tile([128, 64], b16, tag="vtp",
                                                   name="vtp")
                                    nc.tensor.transpose(
                                        out=tp[:],
                                        in_=vt[64 * kvh:64 * (kvh + 1),
                                               128 * cc:128 * (cc + 1)],
                                        identity=identv_t[64 * kvh:64 * (kvh + 1), :])
                                    nc.vector.tensor_copy(
                                        out=VN[kc][:, 65 * kv:65 * kv + 64],
                                        in_=tp[:])
                    for kc in range(NCH):
                        for kv in range(4):
                            nc.vector.memset(VN[kc][:, 65 * kv + 64:65 * kv + 65],
                                             1.0)
                    psBt_cm.__exit__(None, None, None)
                    psB_cm.__exit__(None, None, None)

                # ---- scores^T -> exp -> ctx^T (kv-grouped, static padded) ----
                psC_cm = tc.tile_pool(name="psC", bufs=2, space="PSUM")
                psC = psC_cm.__enter__()
                psD_cm = tc.tile_pool(name="psD", bufs=1, space="PSUM")
                psD = psD_cm.__enter__()
                psT_cm = tc.tile_pool(name="psT", bufs=1, space="PSUM")
                psT = psT_cm.__enter__()
                for kv in range(4):
                    mk, rk = kv // 2, (kv % 2) * 64
                    ctx_pss = [psD.tile([65, 512], f32, tag=f"ctx{hh}",
                                        name=f"ctx{hh}") for hh in range(4)]
                    for kt in range(NCH):
                        j = kt // 4
                        qs = 128 * j
                        pts = []
                        for hh in range(4):
                            h = 4 * kv + hh
                            mq, rq = POS[h] // 2, (POS[h] % 2) * 64
                            s_ps = psC.tile([128, 512], f32, tag="s_ps",
                                            name="s_ps")
                            nc.tensor.matmul(
                                out=s_ps[:, qs:],
                                lhsT=KT_[mk][rk:rk + 64, 128 * kt:128 * (kt + 1)],
                                rhs=QT[mq][rq:rq + 64, qs:], start=True, stop=True)
                            pt = ptp.tile([128, 512], b16, tag="pt", name="pt")
                            nc.scalar.activation(out=pt[:, qs:], in_=s_ps[:, qs:],
                                                 func=AF.Exp)
                            nc.vector.tensor_tensor(
                                out=pt[:, qs:qs + 128], in0=pt[:, qs:qs + 128],
                                in1=mm_t[j][kt % 4][:], op=ALU.mult)
                            pts.append(pt)
                        for hh in range(4):
                            nc.tensor.matmul(
                                out=ctx_pss[hh][:, qs:],
                                lhsT=VN[kt][:, 65 * kv:65 * (kv + 1)],
                                rhs=pts[hh][:, qs:], start=(kt == 0),
                                stop=(kt == NCH - 1), skip_group_check=True)
                    for hh in range(4):
                        h = 4 * kv + hh
                        mq, rq = POS[h] // 2, (POS[h] % 2) * 64
                        dsb = tmp.tile([1, 512], f32, tag="dsb", name="dsb")
                        nc.scalar.copy(dsb[:], ctx_pss[hh][64:65, :])
                        dT = psT.tile([128, 4], f32, tag="dT", name="dT")
                        for j in range(4):
                            nc.tensor.transpose(
                                out=dT[:, j:j + 1],
                                in_=dsb[:, 128 * j:128 * (j + 1)],
                                identity=identf_t[0:1, 0:1])
                        recT = tmp.tile([128, 4], f32, tag="recT", name="recT")
                        nc.vector.reciprocal(recT[:], dT[:])
                        drp = psT.tile([4, 128], f32, tag="drp", name="drp")
                        nc.tensor.transpose(out=drp[:], in_=recT[:],
                                            identity=identf_t[:])
                        drs = tmp.tile([4, 128], f32, tag="drs", name="drs")
                        nc.vector.tensor_copy(out=drs[:], in_=drp[:])
                        nc.sync.dma_start(out=den_scr.ap()[h], in_=drs[:])
                        rbc = tmp.tile([64, 512], f32, tag="rbc", name="rbc")
                        bcast_row(rbc[:], den_scr, 512, 64, base_off=h * 512)
                        nc.vector.tensor_tensor(out=CTX[mq][rq:rq + 64, :],
                                                in0=ctx_pss[hh][0:64, :],
                                                in1=rbc[:], op=ALU.mult)
                psT_cm.__exit__(None, None, None)
                psD_cm.__exit__(None, None, None)
                psC_cm.__exit__(None, None, None)

                # ---- attn out + residual + clip -> h2T ----
                with tc.tile_pool(name="aout", bufs=1) as op_, \
                     tc.tile_pool(name="psE", bufs=2, space="PSUM") as psE:
                    WO = [op_.tile([128, D], b16, tag=f"WO{kt}", name=f"WO{kt}")
                          for kt in range(DT)]
                    XR = [op_.tile([128, 512], f32, tag=f"XR{kt}", name=f"XR{kt}")
                          for kt in range(DT)]
                    for kt in range(DT):
                        nc.sync.dma_start(out=WO[kt][:],
                                          in_=wo.ap()[128 * kt:128 * (kt + 1)])
                        nc.sync.dma_start(out=XR[kt][:],
                                          in_=xTr.ap()[128 * kt:128 * (kt + 1)])
                    for m in range(DT):
                        ao_ps = psE.tile([128, 512], f32, tag="ao_ps", name="ao_ps")
                        for kt in range(DT):
                            nc.tensor.matmul(out=ao_ps[:],
                                             lhsT=WO[kt][:, 128 * m:128 * (m + 1)],
                                             rhs=CTX[kt][:], start=(kt == 0),
                                             stop=(kt == DT - 1))
                        nc.vector.tensor_tensor(out=h2T[m][:], in0=ao_ps[:],
                                                in1=XR[m][:], op=ALU.add)
                        nc.vector.tensor_scalar(out=h2T[m][:], in0=h2T[m][:],
                                                scalar1=100.0, scalar2=-100.0,
                                                op0=ALU.min, op1=ALU.max)

            # ================= rmsnorm2 + routing + AG =================
            with tc.tile_pool(name="moe1", bufs=1) as mp, \
                 tc.tile_pool(name="mtmp", bufs=3) as mt:
                psF_cm = tc.tile_pool(name="psF", bufs=2, space="PSUM")
                psF = psF_cm.__enter__()

                hns_cm = tc.tile_pool(name="hns", bufs=1)
                hns = hns_cm.__enter__()
                hnT = [hns.tile([128, 512], b16, tag=f"hnT{kt}", name=f"hnT{kt}")
                       for kt in range(DT)]
                ssqP2 = psF.tile([128, DT * 4], f32, tag="ssqP2", name="ssqP2")
                for kt in range(DT):
                    x2m = mt.tile([128, 512], b16, tag="x2m", name="x2m")
                    nc.scalar.activation(out=x2m[:], in_=h2T[kt][:], func=AF.Square)
                    for j in range(4):
                        nc.tensor.matmul(out=ssqP2[:, kt * 4 + j:kt * 4 + j + 1],
                                         lhsT=x2m[:, 128 * j:128 * (j + 1)],
                                         rhs=ones[:], start=True, stop=True)
                rstdT2 = mp.tile([128, 4], f32, tag="rstdT2", name="rstdT2")
                nc.vector.tensor_reduce(
                    out=rstdT2[:],
                    in_=bass.AP(tensor=ssqP2[:].tensor, offset=ssqP2[:].offset,
                                ap=[ssqP2[:].ap[0], [1, 4], [4, DT]]),
                    axis=mybir.AxisListType.X, op=ALU.add)
                nc.vector.tensor_scalar(out=rstdT2[:], in0=rstdT2[:],
                                        scalar1=1.0 / D, scalar2=EPS,
                                        op0=ALU.mult, op1=ALU.add)
                nc.scalar.sqrt(rstdT2[:], rstdT2[:])
                nc.vector.reciprocal(rstdT2[:], rstdT2[:])
                rrp2 = psF.tile([4, 128], f32, tag="rrp2", name="rrp2")
                nc.tensor.transpose(out=rrp2[:], in_=rstdT2[:],
                                    identity=identf_t[:])
                rrs2 = mp.tile([4, 128], f32, tag="rrs2", name="rrs2")
                nc.vector.tensor_copy(out=rrs2[:], in_=rrp2[:])
                nc.sync.dma_start(out=rstd2_scr.ap(), in_=rrs2[:])
                rbc2 = mp.tile([128, 512], f32, tag="rstd2bc", name="rstd2bc")
                bcast_row(rbc2[:], rstd2_scr, 512, 128)
                for kt in range(DT):
                    nc.vector.tensor_tensor(out=hnT[kt][:], in0=h2T[kt][:],
                                            in1=rbc2[:], op=ALU.mult)
                if DBG:
                    for kt in range(DT):
                        nc.sync.dma_start(out=dbg["h2T"].ap()[128 * kt:128 * (kt + 1)],
                                          in_=h2T[kt][:])

                # h2 -> N layout (bf16) for the final combine
                for tcn in range(4):
                    for kt in range(DT):
                        h2b = mt.tile([128, 128], b16, tag="h2b", name="h2b")
                        nc.vector.tensor_copy(
                            out=h2b[:], in_=h2T[kt][:, 128 * tcn:128 * (tcn + 1)])
                        tp = psF.tile([128, 128], b16, tag="hntp", name="h2tp")
                        nc.tensor.transpose(out=tp[:], in_=h2b[:],
                                            identity=ident[:])
                        nc.vector.tensor_copy(
                            out=h2N[tcn][:, 128 * kt:128 * (kt + 1)], in_=tp[:])

                WR = mp.tile([128, DT * E], b16, tag="WR", name="WR")
                for kt in range(DT):
                    nc.sync.dma_start(out=WR[:, E * kt:E * (kt + 1)],
                                      in_=wr.ap()[128 * kt:128 * (kt + 1)])

                aux4 = [mp.tile([128, 4], f32, tag=f"aux{tcn}", name=f"aux{tcn}")
                        for tcn in range(4)]
                for tcn in range(4):
                    lg_ps = psF.tile([128, E], f32, tag="lg_ps", name="lg_ps")
                    for kt in range(DT):
                        nc.tensor.matmul(
                            out=lg_ps[:],
                            lhsT=hnT[kt][:, 128 * tcn:128 * (tcn + 1)],
                            rhs=WR[:, E * kt:E * (kt + 1)],
                            start=(kt == 0), stop=(kt == DT - 1))
                    lgn = mt.tile([128, E], f32, tag="lgn", name="lgn")
                    nc.vector.tensor_copy(out=lgn[:], in_=lg_ps[:])
                    m8 = mt.tile([128, 8], f32, tag="m8", name="m8")
                    nc.vector.max(m8[:], lgn[:])
                    i8 = mt.tile([128, 8], u32, tag="i8", name="i8")
                    nc.vector.max_index(i8[:], m8[:], lgn[:])
                    nc.vector.tensor_copy(out=aux4[tcn][:, 0:2], in_=i8[:, 0:2])
                    d12 = mt.tile([128, 1], f32, tag="d12", name="d12")
                    nc.vector.tensor_tensor(out=d12[:], in0=m8[:, 0:1],
                                            in1=m8[:, 1:2], op=ALU.subtract)
                    nc.scalar.activation(out=aux4[tcn][:, 2:3], in_=d12[:],
                                         func=AF.Sigmoid)
                    nc.vector.tensor_scalar(out=aux4[tcn][:, 3:4],
                                            in0=aux4[tcn][:, 2:3],
                                            scalar1=-1.0, scalar2=1.0,
                                            op0=ALU.mult, op1=ALU.add)

                for tcn in range(4):
                    hnn = mt.tile([128, DA], b16, tag="hnn", name="hnn")
                    for kt in range(DT):
                        tp = psF.tile([128, 128], b16, tag="hntp", name="hntp")
                        nc.tensor.transpose(
                            out=tp[:], in_=hnT[kt][:, 128 * tcn:128 * (tcn + 1)],
                            identity=ident[:])
                        nc.vector.tensor_copy(out=hnn[:, 128 * kt:128 * (kt + 1)],
                                              in_=tp[:])
                    nc.vector.tensor_copy(out=hnn[:, D:D + 4], in_=aux4[tcn][:])
                    nc.vector.memset(hnn[:, D + 4:DA], 0)
                    nc.sync.dma_start(out=hn_bounce.ap()[128 * tcn:128 * (tcn + 1)],
                                      in_=hnn[:])

                nc.gpsimd.collective_compute(
                    "AllGather", ALU.bypass, replica_groups=RG,
                    ins=[hn_bounce.ap()], outs=[hn_all.ap()[0:T]])

                if DBG:
                    nc.sync.dma_start(out=dbg["hnall"].ap(), in_=hn_all.ap()[0:T])
                # ---- per-half token lists via sparse_gather ----
                at = mp.tile([16, 256, 4], b16, tag="at", name="at")
                nc.sync.dma_start(
                    out=at[:],
                    in_=hn_all.ap()[0:T, D:D + 4].rearrange("(p f) k -> p f k",
                                                            p=16))
                atf = mp.tile([16, 256, 4], f32, tag="atf", name="atf")
                nc.vector.tensor_copy(out=atf[:], in_=at[:])
                eidc = mp.tile([16, 1], f32, tag="eidc", name="eidc")
                nc.sync.dma_start(out=eidc[:], in_=eid.ap())
                hAc = mp.tile([16, 1], f32, tag="hAc", name="hAc")
                nc.sync.dma_start(out=hAc[:], in_=hA.ap())
                hAm1c = mp.tile([16, 1], f32, tag="hAm1c", name="hAm1c")
                nc.sync.dma_start(out=hAm1c[:], in_=hAm1.ap())
                hBc = mp.tile([16, 1], f32, tag="hBc", name="hBc")
                nc.sync.dma_start(out=hBc[:], in_=hB.ap())
                hBm1c = mp.tile([16, 1], f32, tag="hBm1c", name="hBm1c")
                nc.sync.dma_start(out=hBm1c[:], in_=hBm1.ap())
                sel1 = mp.tile([16, 256], f32, tag="sel1", name="sel1")
                nc.vector.tensor_scalar(out=sel1[:], in0=atf[:, :, 0],
                                        scalar1=eidc[:], scalar2=None,
                                        op0=ALU.is_equal)
                sel2 = mp.tile([16, 256], f32, tag="sel2", name="sel2")
                nc.vector.tensor_scalar(out=sel2[:], in0=atf[:, :, 1],
                                        scalar1=eidc[:], scalar2=None,
                                        op0=ALU.is_equal)
                selt = mp.tile([16, 256], f32, tag="selt", name="selt")
                nc.vector.tensor_tensor(out=selt[:], in0=sel1[:], in1=sel2[:],
                                        op=ALU.max)
                we = mp.tile([16, 256], f32, tag="we", name="we")
                t1_ = mp.tile([16, 256], f32, tag="t1_", name="t1_")
                nc.vector.tensor_tensor(out=we[:], in0=sel1[:], in1=atf[:, :, 2],
                                        op=ALU.mult)
                nc.vector.tensor_tensor(out=t1_[:], in0=sel2[:], in1=atf[:, :, 3],
                                        op=ALU.mult)
                nc.vector.tensor_tensor(out=we[:], in0=we[:], in1=t1_[:], op=ALU.add)
                nc.vector.tensor_scalar(out=we[:], in0=we[:], scalar1=0.9985,
                                        scalar2=1e-4, op0=ALU.min, op1=ALU.max)
                iotat = mp.tile([16, 256], f32, tag="iotat", name="iotat")
                nc.gpsimd.iota(iotat.bitcast(i32)[:], pattern=[[1, 256]], base=0,
                               channel_multiplier=256)
                nc.vector.tensor_copy(out=iotat[:], in_=iotat.bitcast(i32)[:])
                val = mp.tile([16, 256], f32, tag="val", name="val")
                nc.vector.tensor_scalar(out=val[:], in0=we[:], scalar1=0.249,
                                        scalar2=0.25, op0=ALU.mult, op1=ALU.add)
                nc.vector.tensor_scalar(out=t1_[:], in0=sel1[:], scalar1=0.25,
                                        scalar2=None, op0=ALU.mult)
                nc.vector.tensor_tensor(out=val[:], in0=val[:], in1=t1_[:], op=ALU.add)
                nc.vector.tensor_tensor(out=val[:], in0=val[:], in1=iotat[:],
                                        op=ALU.add)
                nc.vector.tensor_tensor(out=val[:], in0=val[:], in1=selt[:],
                                        op=ALU.mult)
                nc.vector.tensor_scalar(out=t1_[:], in0=selt[:], scalar1=-1.0,
                                        scalar2=1.0, op0=ALU.mult, op1=ALU.add)
                nc.vector.tensor_tensor(out=val[:], in0=val[:], in1=t1_[:],
                                        op=ALU.subtract)
                iotas = mp.tile([16, 256], f32, tag="iotas", name="iotas")
                nc.gpsimd.iota(iotas.bitcast(i32)[:], pattern=[[16, 256]], base=0,
                               channel_multiplier=1)
                nc.vector.tensor_copy(out=iotas[:], in_=iotas.bitcast(i32)[:])
                trash = mp.tile([16, 256], f32, tag="trash", name="trash")
                nc.vector.memset(trash[:], float(T + 512) + 0.25)

                idxc = {}
                scatc = {}
                wcc = {}
                for hf, (hc, hm1) in (("A", (hAc, hAm1c)), ("B", (hBc, hBm1c))):
                    valh = mp.tile([16, 256], f32, tag="val_h", name=f"val{hf}")
                    nc.vector.tensor_scalar(out=valh[:], in0=val[:], scalar1=hc[:],
                                            scalar2=None, op0=ALU.mult)
                    nc.vector.tensor_scalar(out=valh[:], in0=valh[:],
                                            scalar1=hm1[:], scalar2=None,
                                            op0=ALU.add)
                    vals = mp.tile([16, 256], f32, tag="vals_h",
                                   name=f"vals{hf}")
                    cnt = mp.tile([1, 1], u32, tag="cnt_h", name=f"cnt{hf}")
                    nc.gpsimd.sparse_gather(out=vals[:], in_=valh[:],
                                            num_found=cnt[:])
                    cntf = mp.tile([1, 1], f32, tag="cntf_h", name=f"cntf{hf}")
                    nc.vector.tensor_copy(out=cntf[:], in_=cnt[:])
                    cntbc = mp.tile([16, 1], f32, tag="cntbc_h",
                                    name=f"cntbc{hf}")
                    nc.gpsimd.partition_broadcast(cntbc[:], cntf[:])
                    padm = mp.tile([16, 256], i32, tag="padm_h",
                                   name=f"padm{hf}")
                    nc.vector.tensor_scalar(out=padm[:], in0=iotas[:],
                                            scalar1=cntbc[:], scalar2=None,
                                            op0=ALU.is_ge)
                    nc.vector.copy_predicated(out=vals[:], mask=padm[:],
                                              data=trash[:])
                    if DBG:
                        if hf == "A":
                            nc.sync.dma_start(out=dbg["vals"].ap(), in_=vals[:])
                            nc.sync.dma_start(out=dbg["cnt"].ap(), in_=cnt[:])
                    NH16 = CAPH // 16
                    toki = mp.tile([16, 256], i32, tag="toki_h",
                                   name=f"toki{hf}")
                    vh = mp.tile([16, 256], f32, tag="vh_h", name=f"vh{hf}")
                    nc.vector.tensor_scalar(out=vh[:], in0=vals[:], scalar1=-0.5,
                                            scalar2=None, op0=ALU.add)
                    nc.vector.tensor_copy(out=toki[:], in_=vh[:])
                    tokf = mp.tile([16, 256], f32, tag="tokf_h",
                                   name=f"tokf{hf}")
                    nc.vector.tensor_copy(out=tokf[:], in_=toki[:])
                    frac = mp.tile([16, 256], f32, tag="frac_h",
                                   name=f"frac{hf}")
                    nc.vector.tensor_tensor(out=frac[:], in0=vals[:], in1=tokf[:],
                                            op=ALU.subtract)
                    prim = mp.tile([16, 256], f32, tag="prim_h",
                                   name=f"prim{hf}")
                    nc.vector.tensor_scalar(out=prim[:], in0=frac[:],
                                            scalar1=0.4999, scalar2=None,
                                            op0=ALU.is_ge)
                    wsl = mp.tile([16, 256], f32, tag="wsl_h", name=f"wsl{hf}")
                    nc.vector.tensor_scalar(out=t1_[:], in0=prim[:], scalar1=0.25,
                                            scalar2=0.25, op0=ALU.mult, op1=ALU.add)
                    nc.vector.tensor_tensor(out=wsl[:], in0=frac[:], in1=t1_[:],
                                            op=ALU.subtract)
                    nc.vector.tensor_scalar(out=wsl[:], in0=wsl[:],
                                            scalar1=1.0 / 0.249, scalar2=None,
                                            op0=ALU.mult)
                    # scatter index: tok - 256*floor(tok/512) (- 256 for half B)
                    q512 = mp.tile([16, 256], f32, tag="q512_h",
                                   name=f"q512{hf}")
                    nc.vector.tensor_scalar(out=q512[:], in0=tokf[:],
                                            scalar1=1.0 / 512, scalar2=-0.499,
                                            op0=ALU.mult, op1=ALU.add)
                    q512i = mp.tile([16, 256], i32, tag="q512i_h",
                                    name=f"q512i{hf}")
                    nc.vector.tensor_copy(out=q512i[:], in_=q512[:])
                    nc.vector.tensor_copy(out=q512[:], in_=q512i[:])
                    scat = mp.tile([16, 256], f32, tag="scat_h",
                                   name=f"scat{hf}")
                    off = -256.0 if hf == "B" else 0.0
                    nc.vector.tensor_scalar(out=scat[:], in0=q512[:],
                                            scalar1=-256.0, scalar2=off,
                                            op0=ALU.mult, op1=ALU.add)
                    nc.vector.tensor_tensor(out=scat[:], in0=scat[:], in1=tokf[:],
                                            op=ALU.add)
                    scati = mp.tile([16, 256], i32, tag="scati_h",
                                    name=f"scati{hf}")
                    nc.vector.tensor_copy(out=scati[:], in_=scat[:])
                    idxc[hf] = [mp.tile([128, 1], i32, tag=f"ix{hf}{st}",
                                        name=f"ix{hf}{st}") for st in range(NSTH)]
                    scatc[hf] = [mp.tile([128, 1], i32, tag=f"sc{hf}{st}",
                                         name=f"sc{hf}{st}") for st in range(NSTH)]
                    wcc[hf] = [mp.tile([128, 1], f32, tag=f"wc{hf}{st}",
                                       name=f"wc{hf}{st}") for st in range(NSTH)]
                    for st in range(NSTH):
                        nc.sync.dma_start(out=idxc[hf][st][:],
                                          in_=toki[:, 8 * st:8 * (st + 1), None])
                        nc.sync.dma_start(out=scatc[hf][st][:],
                                          in_=scati[:, 8 * st:8 * (st + 1), None])
                        nc.sync.dma_start(out=wcc[hf][st][:],
                                          in_=wsl[:, 8 * st:8 * (st + 1), None])
                psF_cm.__exit__(None, None, None)
                hns_cm.__exit__(None, None, None)

                # ============ expert compute (two halves, overlapped RS) ======
                with tc.tile_pool(name="moe2", bufs=1) as ep, \
                     tc.tile_pool(name="etmp", bufs=2) as et, \
                     tc.tile_pool(name="psG", bufs=2, space="PSUM") as psG, \
                     tc.tile_pool(name="psH", bufs=3, space="PSUM") as psH:

                    WG = [ep.tile([128, F], b16, tag=f"WG{kt}", name=f"WG{kt}")
                          for kt in range(DT)]
                    WU = [ep.tile([128, F], b16, tag=f"WU{kt}", name=f"WU{kt}")
                          for kt in range(DT)]
                    for kt in range(DT):
                        nc.sync.dma_start(out=WG[kt][:],
                                          in_=wg.ap()[128 * kt:128 * (kt + 1)])
                        nc.sync.dma_start(out=WU[kt][:],
                                          in_=wu.ap()[128 * kt:128 * (kt + 1)])
                    WD = [ep.tile([128, 512], b16, tag=f"WD{i}", name=f"WD{i}")
                          for i in range(2 * FT)]
                    for dc in range(2):
                        for ft in range(FT):
                            nc.sync.dma_start(
                                out=WD[dc * FT + ft][:],
                                in_=wd.ap()[128 * ft:128 * (ft + 1),
                                            512 * dc:512 * (dc + 1)])

                    XeT = [ep.tile([128, 512], b16, tag=f"XeT{kt}", name=f"XeT{kt}")
                           for kt in range(DT)]
                    aTc = [ep.tile([128, 512], b16, tag=f"aTc{ft}", name=f"aTc{ft}")
                           for ft in range(FT)]

                    def gather_chunk(hf, c0, cw):
                        nst_c = cw // 128
                        for stl in range(nst_c):
                            st = c0 // 128 + stl
                            gr = et.tile([128, DA], b16, tag="gr", name="gr", bufs=1)
                            nc.gpsimd.indirect_dma_start(
                                out=gr[:], out_offset=None, in_=hn_all.ap(),
                                in_offset=bass.IndirectOffsetOnAxis(
                                    ap=idxc[hf][st][:, :1], axis=0),
                                bounds_check=T, oob_is_err=False)
                            for kt in range(DT):
                                tp = psG.tile([128, 128], b16, tag="tp",
                                              name="getp")
                                nc.tensor.transpose(
                                    out=tp[:],
                                    in_=gr[:, 128 * kt:128 * (kt + 1)],
                                    identity=ident[:])
                                nc.vector.tensor_copy(
                                    out=XeT[kt][:, 128 * stl:128 * (stl + 1)],
                                    in_=tp[:])

                    def compute_chunk(hf, ptensor, c0, cw):
                        nst_c = cw // 128
                        for ft in range(FT):
                            g_ps = psH.tile([128, 512], f32, tag="g_ps",
                                            name="g_ps")
                            u_ps = psH.tile([128, 512], f32, tag="u_ps",
                                            name="u_ps")
                            for kt in range(DT):
                                nc.tensor.matmul(
                                    out=g_ps[:, :cw],
                                    lhsT=WG[kt][:, 128 * ft:128 * (ft + 1)],
                                    rhs=XeT[kt][:, :cw],
                                    start=(kt == 0), stop=(kt == DT - 1))
                                nc.tensor.matmul(
                                    out=u_ps[:, :cw],
                                    lhsT=WU[kt][:, 128 * ft:128 * (ft + 1)],
                                    rhs=XeT[kt][:, :cw],
                                    start=(kt == 0), stop=(kt == DT - 1))
                            sg = et.tile([128, 512], b16, tag="sg", name="sg", bufs=1)
                            nc.scalar.activation(out=sg[:, :cw],
                                                 in_=g_ps[:, :cw],
                                                 func=AF.Silu)
                            nc.vector.tensor_tensor(out=aTc[ft][:, :cw],
                                                    in0=sg[:, :cw],
                                                    in1=u_ps[:, :cw],
                                                    op=ALU.mult)
                        for stl in range(nst_c):
                            st = c0 // 128 + stl
                            orow = et.tile([128, D], b16, tag="orow",
                                           name="orow")
                            for dc in range(2):
                                o_ps = psH.tile([128, 512], f32, tag="g_ps",
                                                name="o_ps")
                                for ft in range(FT):
                                    nc.tensor.matmul(
                                        out=o_ps[:],
                                        lhsT=aTc[ft][:,
                                                     128 * stl:128 * (stl + 1)],
                                        rhs=WD[dc * FT + ft][:],
                                        start=(ft == 0), stop=(ft == FT - 1))
                                nc.vector.tensor_scalar(
                                    out=orow[:, 512 * dc:512 * (dc + 1)],
                                    in0=o_ps[:], scalar1=wcc[hf][st][:],
                                    scalar2=None, op0=ALU.mult)
                            nc.gpsimd.indirect_dma_start(
                                out=ptensor.ap(),
                                out_offset=bass.IndirectOffsetOnAxis(
                                    ap=scatc[hf][st][:, :1], axis=0),
                                in_=orow[:], in_offset=None,
                                bounds_check=T // 2 - 1, oob_is_err=False)

                    for (c0, cw) in SCH_H:
                        gather_chunk("A", c0, cw)
                        compute_chunk("A", partA, c0, cw)
                    gather_chunk("B", 0, 512)
                    if DBG:
                        nc.sync.dma_start(out=dbg["part"].ap()[0:T // 2],
                                          in_=partA.ap()[0:T // 2])
                    nc.gpsimd.collective_compute(
                        "ReduceScatter", ALU.add, replica_groups=RG,
                        ins=[partA.ap()[0:T // 2]], outs=[rsA.ap()])
                    compute_chunk("B", partB, 0, 512)
                    gather_chunk("B", 512, 128)
                    compute_chunk("B", partB, 512, 128)
                    if DBG:
                        nc.sync.dma_start(out=dbg["part"].ap()[T // 2:T],
                                          in_=partB.ap()[0:T // 2])
                    nc.gpsimd.collective_compute(
                        "ReduceScatter", ALU.add, replica_groups=RG,
                        ins=[partB.ap()[0:T // 2]], outs=[rsB.ap()])

                    for tcn in range(4):
                        rsrc = rsA if tcn < 2 else rsB
                        roff = 128 * (tcn % 2)
                        mo = et.tile([128, D], b16, tag="mo", name="mo", bufs=1)
                        nc.sync.dma_start(out=mo[:],
                                          in_=rsrc.ap()[roff:roff + 128])
                        for fc in range(2):
                            fin = et.tile([128, 512], f32, tag="fin", name="fin",
                                          bufs=1)
                            nc.vector.tensor_tensor(
                                out=fin[:], in0=mo[:, 512 * fc:512 * (fc + 1)],
                                in1=h2N[tcn][:, 512 * fc:512 * (fc + 1)],
                                op=ALU.add)
                            nc.vector.tensor_scalar(out=fin[:], in0=fin[:],
                                                    scalar1=100.0, scalar2=-100.0,
                                                    op0=ALU.min, op1=ALU.max)
                            nc.sync.dma_start(
                                out=out.ap()[128 * tcn:128 * (tcn + 1),
                                             512 * fc:512 * (fc + 1)],
                                in_=fin[:])

    nc.finalize()
    return nc


@functools.lru_cache(maxsize=1)
def _compiled():
    _hook_init()
    return build()


def _prep(inputs):
    """Host-side input prep -> per-core in_maps."""
    x = np.asarray(inputs["hidden_states"], np.float32)
    ln1 = np.asarray(inputs["ln1_w"], np.float32)
    ln2 = np.asarray(inputs["ln2_w"], np.float32)
    Wq = np.asarray(inputs["Wq"], np.float32) * ln1[:, None]
    Wk = np.asarray(inputs["Wk"], np.float32) * ln1[:, None]
    Wv = np.asarray(inputs["Wv"], np.float32) * ln1[:, None]
    Wo = np.asarray(inputs["Wo"], np.float32)
    Wr = np.asarray(inputs["Wr"], np.float32) * ln2[:, None]
    Wg = np.asarray(inputs["Wg"], np.float32) * ln2[None, :, None]
    Wu = np.asarray(inputs["Wu"], np.float32) * ln2[None, :, None]
    Wd = np.asarray(inputs["Wd"], np.float32)

    inv = (1.0 / (THETA ** (np.arange(0, HD, 2, dtype=np.float32) / HD)))
    pos = np.arange(S, dtype=np.float32)[:, None] * inv[None, :]   # [S, 32]
    cos32, sin32 = np.cos(pos).T, np.sin(pos).T                    # [32, S]
    ck = np.tile(cos32, (4, 1)).astype(bf16)                       # [128, S]
    sk_half = np.concatenate([-sin32, sin32], axis=0)              # [64, S]
    sk = np.tile(sk_half, (2, 1)).astype(bf16)

    xT = [np.ascontiguousarray(x[b].T) for b in range(B)]          # [D, S] f32

    ident = np.eye(128, dtype=bf16)
    identf_ = np.eye(128, dtype=np.float32)
    identv_ = np.vstack([np.eye(64), np.eye(64)]).astype(bf16)
    onesb_ = np.ones((128, 1), dtype=bf16)
    perm_heads = [0, 4, 1, 5, 2, 6, 3, 7, 8, 12, 9, 13, 10, 14, 11, 15]
    colperm = np.concatenate([np.arange(64 * h, 64 * (h + 1)) for h in perm_heads])
    Wq = np.ascontiguousarray(Wq[:, colperm])
    Wo = np.ascontiguousarray(Wo[colperm, :])

    tri = np.triu(np.ones((128, 128), np.float32))                 # k<=q keep
    hA_ = (np.arange(16)[:, None] % 2 == 0).astype(np.float32)
    hB_ = 1.0 - hA_

    in_maps = []
    for c in range(N_CORES):
        bi, qi = c // 4, c % 4
        gs = [qi + 4 * j for j in range(4)]
        cols = np.concatenate([np.arange(128 * g, 128 * (g + 1)) for g in gs])
        mm = np.zeros((4, 4, 128, 128), np.float32)
        for j in range(4):
            for r in range(4):
                if r < qi:
                    mm[j, r] = 1.0
                elif r == qi:
                    mm[j, r] = tri
        in_maps.append({
            "xT": xT[bi].astype(bf16),
            "xTq": np.ascontiguousarray(xT[bi][:, cols]).astype(bf16),
            "xTr": np.ascontiguousarray(xT[bi][:, cols]),
            "wq": Wq.astype(bf16), "wk": Wk.astype(bf16),
            "wv": Wv.astype(bf16), "wo": Wo.astype(bf16),
            "wr": Wr.astype(bf16),
            "wg": Wg[c].astype(bf16), "wu": Wu[c].astype(bf16),
            "wd": Wd[c].astype(bf16),
            "cosq": np.ascontiguousarray(ck[:, cols]),
            "sinq": np.ascontiguousarray(sk[:, cols]),
            "cosk": ck, "sink": sk,
            "maskm": mm.astype(bf16),
            "identb": ident, "identv": identv_, "identf": identf_, "onesb": onesb_,
            "eid": np.full((16, 1), float(c), np.float32),
            "hA": hA_, "hAm1": hA_ - 1.0, "hB": hB_, "hBm1": hB_ - 1.0,
        })
    return in_maps


def kernel(**inputs):
    import os
    from concourse.bass_utils import run_bass_kernel_spmd
    nc = _compiled()
    in_maps = _prep(inputs)
    trace = os.environ.get("KERNEL_TRACE", "0") == "1"
    r = run_bass_kernel_spmd(nc, in_maps, list(range(N_CORES)), trace=trace)
    kernel._results = r
    out = np.empty((B, S, D), np.float32)
    for c in range(N_CORES):
        bi, qi = c // 4, c % 4
        o = np.asarray(r.results[c]["out"])            # [512, 1024]
        for j in range(4):
            g = qi + 4 * j
            out[bi, 128 * g:128 * (g + 1)] = o[128 * j:128 * (j + 1)]
    return out


kernel._results = None

